# revision 10
# baseline (speedup 1.0000x reference)
"""t-SNE style probability encoder on 8 trn2 cores.

MLP 128->64->32->16->16 (relu x3) producing z [8192,16], then
P = rownorm(1/(1 + sqdist(z, z))).

Sharding: core c owns global rows c*1024:(c+1)*1024. Each core runs
the fp32 MLP only on its own 1024 points, builds bf16 hi/lo splits of
z and sq, AllGathers them (50 rows x 1024 cols bf16 per core), and
computes its own [1024, 8192] block of P against the gathered stack.
Host concatenates the 8 row blocks (no cross-device reduction needed:
row-normalization is per-row).

Phase 2 uses a single K=52 bf16 matmul per [128,512] output block,
with hi/lo bf16 splits emulating fp32 products (z = zhi + zlo,
x*y ~ xh*yh + xl*yh + xh*yl; the missing lo*lo term is negligible):
  L [52,1024] own rows: [-2zhi; -2zlo; -2zhi; 1; 1; sqp1h; sqp1l]
  R [52,8192] gathered: [zhi;   zhi;   zlo; sqh; sql;  1;     1  ]
  sum_k L[k,i]*R[k,j] = (1 + sq_i) + sq_j - 2 z_i.z_j = 1 + dist_ij

Device pipeline per core:
  PE:    fp32 MLP matmuls + sq colsum (own cols); bf16 K=52 aug
         matmuls -> PSUM.
  ACT:   relu/bias epilogues, bf16 casts, -2x scales, rowsum accum.
  DVE:   hi/lo split subtracts, reciprocal_approx_fast PSUM->SBUF.
  GPSIMD: bounce DMAs + AllGather collective, normalize_recip.
  DMA:   xT in (512KB), gather scatter into R, P out (32MB).
"""

import sys

import numpy as np

sys.path.insert(0, "/opt/trn_rl_repo")

N = 8192
DIM = 128
EMB = 16
NCORES = 8
ROWS = N // NCORES  # 1024
KAUG = 52  # 3*EMB hi/lo product rows + sqh/sql/sqp1h/sqp1l scalar rows
GROWS = 50  # gathered rows per core: zhi(16) zhi(16) zlo(16) sqh sql

_CACHE = {}


def _build_program():
    from contextlib import ExitStack

    import concourse.bacc as bacc
    import concourse.tile as tile
    from concourse import mybir

    f32 = mybir.dt.float32
    bf16 = mybir.dt.bfloat16
    AF = mybir.ActivationFunctionType
    Alu = mybir.AluOpType

    nc = bacc.Bacc("TRN2", target_bir_lowering=False, debug=False, num_devices=NCORES)

    xT = nc.declare_dram_parameter("xT", [DIM, ROWS], f32, isOutput=False)
    W1 = nc.declare_dram_parameter("W1", [128, 64], f32, isOutput=False)
    W2 = nc.declare_dram_parameter("W2", [64, 32], f32, isOutput=False)
    W3 = nc.declare_dram_parameter("W3", [32, 16], f32, isOutput=False)
    W4 = nc.declare_dram_parameter("W4", [16, 16], f32, isOutput=False)
    b1 = nc.declare_dram_parameter("b1", [64, 1], f32, isOutput=False)
    b2 = nc.declare_dram_parameter("b2", [32, 1], f32, isOutput=False)
    b3 = nc.declare_dram_parameter("b3", [16, 1], f32, isOutput=False)
    b4 = nc.declare_dram_parameter("b4", [16, 1], f32, isOutput=False)
    out = nc.declare_dram_parameter("out", [ROWS, N], f32, isOutput=True)

    with tile.TileContext(nc) as tc, ExitStack() as ctx:
        consts = ctx.enter_context(tc.tile_pool(name="consts", bufs=1))
        persist = ctx.enter_context(tc.tile_pool(name="persist", bufs=1))
        dram = ctx.enter_context(tc.tile_pool(name="dram", bufs=1, space="DRAM"))

        w1_sb = consts.tile([128, 64], f32)
        w2_sb = consts.tile([64, 32], f32)
        w3_sb = consts.tile([32, 16], f32)
        w4_sb = consts.tile([16, 16], f32)
        b1_sb = consts.tile([64, 1], f32)
        b2_sb = consts.tile([32, 1], f32)
        b3_sb = consts.tile([16, 1], f32)
        b4_sb = consts.tile([16, 1], f32)
        ones_sq = consts.tile([16, 1], f32)
        for drm, sb in [
            (W1, w1_sb), (W2, w2_sb), (W3, w3_sb), (W4, w4_sb),
            (b1, b1_sb), (b2, b2_sb), (b3, b3_sb), (b4, b4_sb),
        ]:
            nc.sync.dma_start(sb[:], drm[:])
        nc.vector.memset(ones_sq[:], 1.0)

        # persistent bf16 aug operands (rows 50/51 of R and 48/49 of L
        # stay at the memset value 1.0)
        R = persist.tile([KAUG, N], bf16)
        L = persist.tile([KAUG, ROWS], bf16)
        nc.vector.memset(R[:], 1.0)
        nc.vector.memset(L[:], 1.0)

        inb = dram.tile([GROWS, ROWS], bf16)
        outb = dram.tile([NCORES * GROWS, ROWS], bf16)
        warm_in = dram.tile([1, 128], bf16)
        warm_out = dram.tile([NCORES, 128], bf16)

        # warm-up collective: absorbs CC rendezvous/setup latency while the
        # MLP runs, so the real AllGather below completes quickly
        nc.gpsimd.collective_compute(
            "AllGather",
            mybir.AluOpType.bypass,
            replica_groups=[list(range(NCORES))],
            ins=[warm_in.opt()],
            outs=[warm_out.opt()],
        )

        # ---------------- Phase 1: MLP on own cols -> splits ----------------
        CH = 512
        with tc.tile_pool(name="zpool", bufs=1) as zpool:
            zT = zpool.tile([EMB, ROWS], f32)
            zhi = zpool.tile([EMB, ROWS], bf16)
            zlo = zpool.tile([EMB, ROWS], bf16)
            sqh = zpool.tile([1, ROWS], bf16)
            sql = zpool.tile([1, ROWS], bf16)
            sp1 = zpool.tile([1, ROWS], f32)  # sq_own + 1 in fp32

            with (
                tc.tile_pool(name="mlp_x", bufs=1) as xpool,
                tc.tile_pool(name="mlp_h", bufs=2) as hpool,
                tc.tile_pool(name="ps1", bufs=2, space="PSUM") as ps1p,
                tc.tile_pool(name="ps2", bufs=1, space="PSUM") as ps2p,
                tc.tile_pool(name="ps3", bufs=1, space="PSUM") as ps3p,
                tc.tile_pool(name="ps4", bufs=1, space="PSUM") as ps4p,
                tc.tile_pool(name="pssq", bufs=1, space="PSUM") as psqp,
            ):
                xt_sb = xpool.tile([DIM, ROWS], f32)
                nc.sync.dma_start(xt_sb[:], xT[:])

                for n in range(ROWS // CH):
                    s = n * CH
                    p1 = ps1p.tile([64, CH], f32, name="p1")
                    nc.tensor.matmul(p1[:], w1_sb[:], xt_sb[:, s:s + CH], start=True, stop=True)
                    h1 = hpool.tile([64, CH], f32, name="h1")
                    nc.scalar.activation(h1[:], p1[:], AF.Relu, bias=b1_sb[:])

                    p2 = ps2p.tile([32, CH], f32, name="p2")
                    nc.tensor.matmul(p2[:], w2_sb[:], h1[:], start=True, stop=True)
                    h2 = hpool.tile([32, CH], f32, name="h2")
                    nc.scalar.activation(h2[:], p2[:], AF.Relu, bias=b2_sb[:])

                    p3 = ps3p.tile([16, CH], f32, name="p3")
                    nc.tensor.matmul(p3[:], w3_sb[:], h2[:], start=True, stop=True)
                    h3 = hpool.tile([16, CH], f32, name="h3")
                    nc.scalar.activation(h3[:], p3[:], AF.Relu, bias=b3_sb[:])

                    p4 = ps4p.tile([16, CH], f32, name="p4")
                    nc.tensor.matmul(p4[:], w4_sb[:], h3[:], start=True, stop=True)
                    nc.scalar.activation(zT[:, s:s + CH], p4[:], AF.Identity, bias=b4_sb[:])
                    nc.scalar.activation(zhi[:, s:s + CH], p4[:], AF.Identity, bias=b4_sb[:])
                    zt2 = hpool.tile([16, CH], f32, name="zt2")
                    nc.scalar.activation(zt2[:], p4[:], AF.Square, bias=b4_sb[:])

                    psq = psqp.tile([1, CH], f32, name="psq")
                    nc.tensor.matmul(psq[:], ones_sq[:], zt2[:], start=True, stop=True)

                    # hi/lo split of z and sq (bf16)
                    nc.vector.scalar_tensor_tensor(
                        zlo[:, s:s + CH], zT[:, s:s + CH], 0.0,
                        zhi[:, s:s + CH], Alu.add, Alu.subtract,
                    )
                    nc.scalar.activation(sqh[0:1, s:s + CH], psq[:], AF.Copy, bias=0.0)
                    nc.vector.scalar_tensor_tensor(
                        sql[0:1, s:s + CH], psq[:], 0.0,
                        sqh[0:1, s:s + CH], Alu.add, Alu.subtract,
                    )
                    nc.scalar.activation(sp1[0:1, s:s + CH], psq[:], AF.Copy, bias=1.0)

            # ---- gather hi/lo splits of all points into R ----
            nc.sync.dma_start(inb[0:EMB, :], zhi[:, :])
            nc.sync.dma_start(inb[EMB:2 * EMB, :], zhi[:, :])
            nc.sync.dma_start(inb[2 * EMB:3 * EMB, :], zlo[:, :])
            nc.sync.dma_start(inb[48:49, :], sqh[:, :])
            nc.sync.dma_start(inb[49:50, :], sql[:, :])
            nc.gpsimd.collective_compute(
                "AllGather",
                mybir.AluOpType.bypass,
                replica_groups=[list(range(NCORES))],
                ins=[inb.opt()],
                outs=[outb.opt()],
            )
            for c in range(NCORES):
                nc.gpsimd.dma_start(
                    R[0:GROWS, c * ROWS:(c + 1) * ROWS],
                    outb[c * GROWS:(c + 1) * GROWS, :],
                )

            # ---- build L from own-col splits ----
            with tc.tile_pool(name="fin", bufs=1) as fin:
                m2zhi = fin.tile([EMB, ROWS], bf16)
                m2zlo = fin.tile([EMB, ROWS], bf16)
                sph = fin.tile([1, ROWS], bf16)
                spl = fin.tile([1, ROWS], bf16)

                nc.scalar.activation(m2zhi[:], zhi[:, :], AF.Copy, bias=0.0, scale=-2.0)
                nc.scalar.activation(m2zlo[:], zlo[:, :], AF.Copy, bias=0.0, scale=-2.0)
                nc.scalar.activation(sph[:], sp1[:], AF.Copy, bias=0.0)
                nc.vector.scalar_tensor_tensor(
                    spl[:], sp1[:], 0.0, sph[:], Alu.add, Alu.subtract
                )
                nc.sync.dma_start(L[0:EMB, :], m2zhi[:])
                nc.sync.dma_start(L[EMB:2 * EMB, :], m2zlo[:])
                nc.sync.dma_start(L[2 * EMB:3 * EMB, :], m2zhi[:])
                nc.sync.dma_start(L[50:51, :], sph[:])
                nc.sync.dma_start(L[51:52, :], spl[:])

        # ------- Phase 2: 1+dist -> recip -> rowsum -> normalize -> out -------
        W = 2048
        NW = N // W  # 4
        with (
            tc.tile_pool(name="acc", bufs=3) as apool,
            tc.tile_pool(name="rs", bufs=3) as rspool,
            tc.tile_pool(name="psA", bufs=2, space="PSUM") as psap,
        ):
            for m in range(NCORES):
                A = apool.tile([128, N], f32, name="A")
                rsum = rspool.tile([128, 1], f32, name="rsum")
                lm = L[:, m * 128:(m + 1) * 128]
                for w in range(NW):
                    ps = psap.tile([128, W], f32, name="ps")
                    for h in range(W // 512):
                        col = w * W + h * 512
                        nc.tensor.matmul(
                            ps[:, h * 512:(h + 1) * 512], lm,
                            R[:, col:col + 512], start=True, stop=True,
                        )
                    # num = 1/(1+dist)
                    nc.vector.reciprocal_approx_fast(
                        A[:, w * W:(w + 1) * W], ps[:]
                    )
                # rowsum via in-place copy with accumulate
                nc.scalar.activation(
                    A[:], A[:], AF.Copy, bias=0.0, accum_out=rsum[:]
                )
                nc.gpsimd.normalize_recip(A[:], A[:], rsum[:])
                for d in range(4):
                    nc.sync.dma_start(
                        out[m * 128:(m + 1) * 128, d * 2048:(d + 1) * 2048],
                        A[:, d * 2048:(d + 1) * 2048],
                    )

    nc.compile()
    return nc


def _get_nc():
    if "nc" not in _CACHE:
        _CACHE["nc"] = _build_program()
    return _CACHE["nc"]


def run(inputs, trace=False):
    from concourse.bass_utils import run_bass_kernel_spmd

    nc = _get_nc()
    x = np.asarray(inputs["x"], dtype=np.float32)
    com = {
        "W1": np.ascontiguousarray(np.asarray(inputs["W1"], dtype=np.float32)),
        "W2": np.ascontiguousarray(np.asarray(inputs["W2"], dtype=np.float32)),
        "W3": np.ascontiguousarray(np.asarray(inputs["W3"], dtype=np.float32)),
        "W4": np.ascontiguousarray(np.asarray(inputs["W4"], dtype=np.float32)),
        "b1": np.ascontiguousarray(np.asarray(inputs["b1"], dtype=np.float32).reshape(-1, 1)),
        "b2": np.ascontiguousarray(np.asarray(inputs["b2"], dtype=np.float32).reshape(-1, 1)),
        "b3": np.ascontiguousarray(np.asarray(inputs["b3"], dtype=np.float32).reshape(-1, 1)),
        "b4": np.ascontiguousarray(np.asarray(inputs["b4"], dtype=np.float32).reshape(-1, 1)),
    }
    in_maps = []
    for c in range(NCORES):
        xT_c = np.ascontiguousarray(x[c * ROWS:(c + 1) * ROWS].T)
        in_maps.append({"xT": xT_c, **com})

    res = run_bass_kernel_spmd(nc, in_maps, core_ids=list(range(NCORES)), trace=trace)
    full = np.concatenate(
        [res.results[c]["out"] for c in range(NCORES)], axis=0
    ).astype(np.float32)
    return full, res


def kernel(**inputs):
    full, _ = run(inputs, trace=False)
    return full


# revision 14
# speedup vs baseline: 1.2122x; 1.2122x over previous
"""t-SNE style probability encoder on 8 trn2 cores.

MLP 128->64->32->16->16 (relu x3) producing z [8192,16], then
P = rownorm(1/(1 + sqdist(z, z))).

Sharding: core c owns global rows c*1024:(c+1)*1024. Each core runs
the fp32 MLP only on its own 1024 points, builds bf16 hi/lo splits of
z and sq, AllGathers them (50 rows x 1024 cols bf16 per core), and
computes its own [1024, 8192] block of P against the gathered stack.
Host concatenates the 8 row blocks (no cross-device reduction needed:
row-normalization is per-row).

Phase 2 uses a single K=52 bf16 matmul per [128,512] output block,
with hi/lo bf16 splits emulating fp32 products (z = zhi + zlo,
x*y ~ xh*yh + xl*yh + xh*yl; the missing lo*lo term is negligible):
  L [52,1024] own rows: [-2zhi; -2zlo; -2zhi; 1; 1; sqp1h; sqp1l]
  R [52,8192] gathered: [zhi;   zhi;   zlo; sqh; sql;  1;     1  ]
  sum_k L[k,i]*R[k,j] = (1 + sq_i) + sq_j - 2 z_i.z_j = 1 + dist_ij

Device pipeline per core:
  PE:    fp32 MLP matmuls + sq colsum (own cols); bf16 K=52 aug
         matmuls -> PSUM.
  ACT:   relu/bias epilogues, bf16 casts, -2x scales, rowsum accum.
  DVE:   hi/lo split subtracts, reciprocal_approx_fast PSUM->SBUF.
  GPSIMD: bounce DMAs + AllGather collective, normalize_recip.
  DMA:   xT in (512KB), gather scatter into R, P out (32MB).
"""

import sys

import numpy as np

sys.path.insert(0, "/opt/trn_rl_repo")

N = 8192
DIM = 128
EMB = 16
NCORES = 8
ROWS = N // NCORES  # 1024
KAUG = 52  # 3*EMB hi/lo product rows + sqh/sql/sqp1h/sqp1l scalar rows
GROWS = 50  # gathered rows per core: zhi(16) zhi(16) zlo(16) sqh sql

_CACHE = {}


def _build_program():
    from contextlib import ExitStack

    import concourse.bacc as bacc
    import concourse.tile as tile
    from concourse import mybir

    f32 = mybir.dt.float32
    f16 = mybir.dt.float16
    bf16 = mybir.dt.bfloat16
    AF = mybir.ActivationFunctionType
    Alu = mybir.AluOpType

    nc = bacc.Bacc("TRN2", target_bir_lowering=False, debug=False, num_devices=NCORES)

    xT = nc.declare_dram_parameter("xT", [DIM, ROWS], f32, isOutput=False)
    W1 = nc.declare_dram_parameter("W1", [128, 64], f32, isOutput=False)
    W2 = nc.declare_dram_parameter("W2", [64, 32], f32, isOutput=False)
    W3 = nc.declare_dram_parameter("W3", [32, 16], f32, isOutput=False)
    W4 = nc.declare_dram_parameter("W4", [16, 16], f32, isOutput=False)
    b1 = nc.declare_dram_parameter("b1", [64, 1], f32, isOutput=False)
    b2 = nc.declare_dram_parameter("b2", [32, 1], f32, isOutput=False)
    b3 = nc.declare_dram_parameter("b3", [16, 1], f32, isOutput=False)
    b4 = nc.declare_dram_parameter("b4", [16, 1], f32, isOutput=False)
    out = nc.declare_dram_parameter("out", [ROWS, N], f16, isOutput=True)

    with tile.TileContext(nc) as tc, ExitStack() as ctx:
        consts = ctx.enter_context(tc.tile_pool(name="consts", bufs=1))
        persist = ctx.enter_context(tc.tile_pool(name="persist", bufs=1))
        dram = ctx.enter_context(tc.tile_pool(name="dram", bufs=1, space="DRAM"))

        w1_sb = consts.tile([128, 64], f32)
        w2_sb = consts.tile([64, 32], f32)
        w3_sb = consts.tile([32, 16], f32)
        w4_sb = consts.tile([16, 16], f32)
        b1_sb = consts.tile([64, 1], f32)
        b2_sb = consts.tile([32, 1], f32)
        b3_sb = consts.tile([16, 1], f32)
        b4_sb = consts.tile([16, 1], f32)
        ones_sq = consts.tile([16, 1], f32)
        for drm, sb in [
            (W1, w1_sb), (W2, w2_sb), (W3, w3_sb), (W4, w4_sb),
            (b1, b1_sb), (b2, b2_sb), (b3, b3_sb), (b4, b4_sb),
        ]:
            nc.sync.dma_start(sb[:], drm[:])
        nc.vector.memset(ones_sq[:], 1.0)

        # persistent bf16 aug operands (rows 50/51 of R and 48/49 of L
        # stay at the memset value 1.0)
        R = persist.tile([KAUG, N], bf16)
        L = persist.tile([KAUG, ROWS], bf16)
        nc.vector.memset(R[:], 1.0)
        nc.vector.memset(L[:], 1.0)

        inb = dram.tile([GROWS, ROWS], bf16)
        outb = dram.tile([NCORES * GROWS, ROWS], bf16)

        # ---------------- Phase 1: MLP on own cols -> splits ----------------
        CH = 512
        with tc.tile_pool(name="zpool", bufs=1) as zpool:
            zT = zpool.tile([EMB, ROWS], f32)
            zhi = zpool.tile([EMB, ROWS], bf16)
            zlo = zpool.tile([EMB, ROWS], bf16)
            sqh = zpool.tile([1, ROWS], bf16)
            sql = zpool.tile([1, ROWS], bf16)
            sp1 = zpool.tile([1, ROWS], f32)  # sq_own + 1 in fp32

            with (
                tc.tile_pool(name="mlp_x", bufs=1) as xpool,
                tc.tile_pool(name="mlp_h", bufs=2) as hpool,
                tc.tile_pool(name="ps1", bufs=2, space="PSUM") as ps1p,
                tc.tile_pool(name="ps2", bufs=1, space="PSUM") as ps2p,
                tc.tile_pool(name="ps3", bufs=1, space="PSUM") as ps3p,
                tc.tile_pool(name="ps4", bufs=1, space="PSUM") as ps4p,
                tc.tile_pool(name="pssq", bufs=1, space="PSUM") as psqp,
            ):
                xt_sb = xpool.tile([DIM, ROWS], f32)
                nc.sync.dma_start(xt_sb[:], xT[:])

                for n in range(ROWS // CH):
                    s = n * CH
                    p1 = ps1p.tile([64, CH], f32, name="p1")
                    nc.tensor.matmul(p1[:], w1_sb[:], xt_sb[:, s:s + CH], start=True, stop=True)
                    h1 = hpool.tile([64, CH], f32, name="h1")
                    nc.scalar.activation(h1[:], p1[:], AF.Relu, bias=b1_sb[:])

                    p2 = ps2p.tile([32, CH], f32, name="p2")
                    nc.tensor.matmul(p2[:], w2_sb[:], h1[:], start=True, stop=True)
                    h2 = hpool.tile([32, CH], f32, name="h2")
                    nc.scalar.activation(h2[:], p2[:], AF.Relu, bias=b2_sb[:])

                    p3 = ps3p.tile([16, CH], f32, name="p3")
                    nc.tensor.matmul(p3[:], w3_sb[:], h2[:], start=True, stop=True)
                    h3 = hpool.tile([16, CH], f32, name="h3")
                    nc.scalar.activation(h3[:], p3[:], AF.Relu, bias=b3_sb[:])

                    p4 = ps4p.tile([16, CH], f32, name="p4")
                    nc.tensor.matmul(p4[:], w4_sb[:], h3[:], start=True, stop=True)
                    nc.scalar.activation(zT[:, s:s + CH], p4[:], AF.Identity, bias=b4_sb[:])
                    nc.scalar.activation(zhi[:, s:s + CH], p4[:], AF.Identity, bias=b4_sb[:])
                    zt2 = hpool.tile([16, CH], f32, name="zt2")
                    nc.scalar.activation(zt2[:], p4[:], AF.Square, bias=b4_sb[:])

                    psq = psqp.tile([1, CH], f32, name="psq")
                    nc.tensor.matmul(psq[:], ones_sq[:], zt2[:], start=True, stop=True)

                    # hi/lo split of z and sq (bf16)
                    nc.vector.scalar_tensor_tensor(
                        zlo[:, s:s + CH], zT[:, s:s + CH], 0.0,
                        zhi[:, s:s + CH], Alu.add, Alu.subtract,
                    )
                    nc.scalar.activation(sqh[0:1, s:s + CH], psq[:], AF.Copy, bias=0.0)
                    nc.vector.scalar_tensor_tensor(
                        sql[0:1, s:s + CH], psq[:], 0.0,
                        sqh[0:1, s:s + CH], Alu.add, Alu.subtract,
                    )
                    nc.scalar.activation(sp1[0:1, s:s + CH], psq[:], AF.Copy, bias=1.0)

            # ---- gather hi/lo splits of all points into R ----
            nc.sync.dma_start(inb[0:EMB, :], zhi[:, :])
            nc.sync.dma_start(inb[EMB:2 * EMB, :], zhi[:, :])
            nc.sync.dma_start(inb[2 * EMB:3 * EMB, :], zlo[:, :])
            nc.sync.dma_start(inb[48:49, :], sqh[:, :])
            nc.sync.dma_start(inb[49:50, :], sql[:, :])
            nc.gpsimd.collective_compute(
                "AllGather",
                mybir.AluOpType.bypass,
                replica_groups=[list(range(NCORES))],
                ins=[inb.opt()],
                outs=[outb.opt()],
            )
            for c in range(NCORES):
                nc.gpsimd.dma_start(
                    R[0:GROWS, c * ROWS:(c + 1) * ROWS],
                    outb[c * GROWS:(c + 1) * GROWS, :],
                )

            # ---- build L from own-col splits ----
            with tc.tile_pool(name="fin", bufs=1) as fin:
                m2zhi = fin.tile([EMB, ROWS], bf16)
                m2zlo = fin.tile([EMB, ROWS], bf16)
                sph = fin.tile([1, ROWS], bf16)
                spl = fin.tile([1, ROWS], bf16)

                nc.scalar.activation(m2zhi[:], zhi[:, :], AF.Copy, bias=0.0, scale=-2.0)
                nc.scalar.activation(m2zlo[:], zlo[:, :], AF.Copy, bias=0.0, scale=-2.0)
                nc.scalar.activation(sph[:], sp1[:], AF.Copy, bias=0.0)
                nc.vector.scalar_tensor_tensor(
                    spl[:], sp1[:], 0.0, sph[:], Alu.add, Alu.subtract
                )
                nc.sync.dma_start(L[0:EMB, :], m2zhi[:])
                nc.sync.dma_start(L[EMB:2 * EMB, :], m2zlo[:])
                nc.sync.dma_start(L[2 * EMB:3 * EMB, :], m2zhi[:])
                nc.sync.dma_start(L[50:51, :], sph[:])
                nc.sync.dma_start(L[51:52, :], spl[:])

        # ------- Phase 2: 1+dist -> recip -> rowsum -> normalize -> out -------
        W = 2048
        NW = N // W  # 4
        with (
            tc.tile_pool(name="acc", bufs=3) as apool,
            tc.tile_pool(name="acc16", bufs=2) as a16pool,
            tc.tile_pool(name="rs", bufs=2) as rspool,
            tc.tile_pool(name="psA", bufs=2, space="PSUM") as psap,
        ):
            for m in range(NCORES):
                A = apool.tile([128, N], f32, name="A")
                A16 = a16pool.tile([128, N], f16, name="A16")
                rs4 = rspool.tile([128, NW], f32, name="rs4")
                junk4 = rspool.tile([128, NW], f32, name="junk4")
                rsum = rspool.tile([128, 1], f32, name="rsum")
                rc = [
                    rspool.tile([128, 1], f32, name=f"rc{w}")
                    for w in range(1, NW)
                ]
                lm = L[:, m * 128:(m + 1) * 128]
                for w in range(NW):
                    ps = psap.tile([128, W], f32, name="ps")
                    for h in range(W // 512):
                        col = w * W + h * 512
                        nc.tensor.matmul(
                            ps[:, h * 512:(h + 1) * 512], lm,
                            R[:, col:col + 512], start=True, stop=True,
                        )
                    # num = 1/(1+dist)
                    nc.vector.reciprocal_approx_fast(
                        A[:, w * W:(w + 1) * W], ps[:]
                    )
                    # partial rowsum of this chunk (in-place copy + accum)
                    nc.scalar.activation(
                        A[:, w * W:(w + 1) * W], A[:, w * W:(w + 1) * W],
                        AF.Copy, bias=0.0, accum_out=rs4[:, w:w + 1],
                    )
                # total rowsum, plus copies (normalize_recip clobbers denom)
                nc.scalar.activation(
                    junk4[:], rs4[:], AF.Copy, bias=0.0, accum_out=rsum[:]
                )
                for w in range(1, NW):
                    nc.scalar.activation(rc[w - 1][:], rsum[:], AF.Copy, bias=0.0)
                for w in range(NW):
                    den = rsum if w == 0 else rc[w - 1]
                    nc.gpsimd.normalize_recip(
                        A16[:, w * W:(w + 1) * W], A[:, w * W:(w + 1) * W],
                        den[:],
                    )
                    nc.sync.dma_start(
                        out[m * 128:(m + 1) * 128, w * W:(w + 1) * W],
                        A16[:, w * W:(w + 1) * W],
                    )

    nc.compile()
    return nc


def _get_nc():
    if "nc" not in _CACHE:
        _CACHE["nc"] = _build_program()
    return _CACHE["nc"]


def run(inputs, trace=False):
    from concourse.bass_utils import run_bass_kernel_spmd

    nc = _get_nc()
    x = np.asarray(inputs["x"], dtype=np.float32)
    com = {
        "W1": np.ascontiguousarray(np.asarray(inputs["W1"], dtype=np.float32)),
        "W2": np.ascontiguousarray(np.asarray(inputs["W2"], dtype=np.float32)),
        "W3": np.ascontiguousarray(np.asarray(inputs["W3"], dtype=np.float32)),
        "W4": np.ascontiguousarray(np.asarray(inputs["W4"], dtype=np.float32)),
        "b1": np.ascontiguousarray(np.asarray(inputs["b1"], dtype=np.float32).reshape(-1, 1)),
        "b2": np.ascontiguousarray(np.asarray(inputs["b2"], dtype=np.float32).reshape(-1, 1)),
        "b3": np.ascontiguousarray(np.asarray(inputs["b3"], dtype=np.float32).reshape(-1, 1)),
        "b4": np.ascontiguousarray(np.asarray(inputs["b4"], dtype=np.float32).reshape(-1, 1)),
    }
    in_maps = []
    for c in range(NCORES):
        xT_c = np.ascontiguousarray(x[c * ROWS:(c + 1) * ROWS].T)
        in_maps.append({"xT": xT_c, **com})

    res = run_bass_kernel_spmd(nc, in_maps, core_ids=list(range(NCORES)), trace=trace)
    full = np.concatenate(
        [res.results[c]["out"] for c in range(NCORES)], axis=0
    ).astype(np.float32)
    return full, res


def kernel(**inputs):
    full, _ = run(inputs, trace=False)
    return full


# revision 21
# speedup vs baseline: 1.3309x; 1.0979x over previous
"""t-SNE style probability encoder on 8 trn2 cores.

MLP 128->64->32->16->16 (relu x3) producing z [8192,16], then
P = rownorm(1/(1 + sqdist(z, z))).

Sharding: core c owns global rows c*1024:(c+1)*1024. Each core runs
the fp32 MLP only on its own 1024 points, builds bf16 hi/lo splits of
z and sq, AllGathers them (50 rows x 1024 cols bf16 per core), and
computes its own [1024, 8192] block of P against the gathered stack.
Host concatenates the 8 row blocks (no cross-device reduction needed:
row-normalization is per-row).

Phase 2 uses a single K=52 bf16 matmul per [128,512] output block,
with hi/lo bf16 splits emulating fp32 products (z = zhi + zlo,
x*y ~ xh*yh + xl*yh + xh*yl; the missing lo*lo term is negligible):
  L [52,1024] own rows: [-2zhi; -2zhi;   1;   1; -2zlo;  sqp1h; sqp1l]
  R [52,8192] gathered: [zhi;   zlo;   sqh; sql; zhi-dup;  1;     1 ]
  sum_k L[k,i]*R[k,j] = (1 + sq_i) + sq_j - 2 z_i.z_j = 1 + dist_ij
Only 34 rows are AllGathered (zhi/zlo/sqh/sql); the zhi dup block is
re-scattered locally from the gather output.

Device pipeline per core:
  PE:    fp32 MLP matmuls + sq colsum (own cols); bf16 K=52 aug
         matmuls -> PSUM.
  ACT:   relu/bias epilogues, bf16 casts, -2x scales, rowsum accum.
  DVE:   hi/lo split subtracts, reciprocal_approx_fast PSUM->SBUF.
  GPSIMD: bounce DMAs + AllGather collective, normalize_recip.
  DMA:   xT in (512KB), gather scatter into R, P out (32MB).
"""

import sys

import numpy as np

sys.path.insert(0, "/opt/trn_rl_repo")

N = 8192
DIM = 128
EMB = 16
NCORES = 8
ROWS = N // NCORES  # 1024
KAUG = 52  # 3*EMB hi/lo product rows + sqh/sql/sqp1h/sqp1l scalar rows
GROWS = 34  # gathered rows per core: zhi(16) zlo(16) sqh sql (zhi dup'd locally)

_CACHE = {}


def _build_program():
    from contextlib import ExitStack

    import concourse.bacc as bacc
    import concourse.tile as tile
    from concourse import mybir

    f32 = mybir.dt.float32
    f16 = mybir.dt.float16
    bf16 = mybir.dt.bfloat16
    AF = mybir.ActivationFunctionType
    Alu = mybir.AluOpType

    nc = bacc.Bacc("TRN2", target_bir_lowering=False, debug=False, num_devices=NCORES)

    xT = nc.declare_dram_parameter("xT", [DIM, ROWS], f32, isOutput=False)
    W1 = nc.declare_dram_parameter("W1", [128, 64], f32, isOutput=False)
    W2 = nc.declare_dram_parameter("W2", [64, 32], f32, isOutput=False)
    W3 = nc.declare_dram_parameter("W3", [32, 16], f32, isOutput=False)
    W4 = nc.declare_dram_parameter("W4", [16, 16], f32, isOutput=False)
    b1 = nc.declare_dram_parameter("b1", [64, 1], f32, isOutput=False)
    b2 = nc.declare_dram_parameter("b2", [32, 1], f32, isOutput=False)
    b3 = nc.declare_dram_parameter("b3", [16, 1], f32, isOutput=False)
    b4 = nc.declare_dram_parameter("b4", [16, 1], f32, isOutput=False)
    out = nc.declare_dram_parameter("out", [ROWS, N], f16, isOutput=True)

    with tile.TileContext(nc) as tc, ExitStack() as ctx:
        consts = ctx.enter_context(tc.tile_pool(name="consts", bufs=1))
        persist = ctx.enter_context(tc.tile_pool(name="persist", bufs=1))
        dram = ctx.enter_context(tc.tile_pool(name="dram", bufs=1, space="DRAM"))

        xt_sb = consts.tile([DIM, ROWS], f32)
        w1_sb = consts.tile([128, 64], f32)
        w2_sb = consts.tile([64, 32], f32)
        w3_sb = consts.tile([32, 16], f32)
        w4_sb = consts.tile([16, 16], f32)
        b1_sb = consts.tile([64, 1], f32)
        b2_sb = consts.tile([32, 1], f32)
        b3_sb = consts.tile([16, 1], f32)
        b4_sb = consts.tile([16, 1], f32)
        ones_sq = consts.tile([16, 1], f32)
        # xT first: the first MLP matmul only needs xt + w1 + b1
        for drm, sb in [
            (xT, xt_sb), (W1, w1_sb), (b1, b1_sb),
            (W2, w2_sb), (W3, w3_sb), (W4, w4_sb),
            (b2, b2_sb), (b3, b3_sb), (b4, b4_sb),
        ]:
            nc.sync.dma_start(sb[:], drm[:])
        nc.vector.memset(ones_sq[:], 1.0)

        # persistent bf16 aug operands; only the constant-one rows need
        # memset (R rows 50/51 pair L's sqp1 rows; L rows 32/33 pair R's
        # sqh/sql rows). Everything else is DMA-written before use.
        R = persist.tile([KAUG, N], bf16)
        L = persist.tile([KAUG, ROWS], bf16)
        # engine ops need partition base 0/32/64/96 -> memset rows 32:52;
        # rows 32:50 are later overwritten by the scatter DMAs.
        nc.vector.memset(R[32:52, :], 1.0)
        nc.vector.memset(L[32:34, :], 1.0)

        inb = dram.tile([GROWS, ROWS], bf16)
        outb = dram.tile([NCORES * GROWS, ROWS], bf16)

        # ---------------- Phase 1: MLP on own cols -> splits ----------------
        CH = 512
        with tc.tile_pool(name="zpool", bufs=1) as zpool:
            zT = zpool.tile([EMB, ROWS], f32)
            zhi = zpool.tile([EMB, ROWS], bf16)
            zlo = zpool.tile([EMB, ROWS], bf16)
            sqh = zpool.tile([1, ROWS], bf16)
            sql = zpool.tile([1, ROWS], bf16)
            sp1 = zpool.tile([1, ROWS], f32)  # sq_own + 1 in fp32

            with (
                tc.tile_pool(name="mlp_h", bufs=2) as hpool,
                tc.tile_pool(name="ps1", bufs=2, space="PSUM") as ps1p,
                tc.tile_pool(name="ps2", bufs=1, space="PSUM") as ps2p,
                tc.tile_pool(name="ps3", bufs=1, space="PSUM") as ps3p,
                tc.tile_pool(name="ps4", bufs=1, space="PSUM") as ps4p,
                tc.tile_pool(name="pssq", bufs=1, space="PSUM") as psqp,
            ):
                for n in range(ROWS // CH):
                    s = n * CH
                    p1 = ps1p.tile([64, CH], f32, name="p1")
                    nc.tensor.matmul(p1[:], w1_sb[:], xt_sb[:, s:s + CH], start=True, stop=True)
                    h1 = hpool.tile([64, CH], f32, name="h1")
                    nc.scalar.activation(h1[:], p1[:], AF.Relu, bias=b1_sb[:])

                    p2 = ps2p.tile([32, CH], f32, name="p2")
                    nc.tensor.matmul(p2[:], w2_sb[:], h1[:], start=True, stop=True)
                    h2 = hpool.tile([32, CH], f32, name="h2")
                    nc.scalar.activation(h2[:], p2[:], AF.Relu, bias=b2_sb[:])

                    p3 = ps3p.tile([16, CH], f32, name="p3")
                    nc.tensor.matmul(p3[:], w3_sb[:], h2[:], start=True, stop=True)
                    h3 = hpool.tile([16, CH], f32, name="h3")
                    nc.scalar.activation(h3[:], p3[:], AF.Relu, bias=b3_sb[:])

                    p4 = ps4p.tile([16, CH], f32, name="p4")
                    nc.tensor.matmul(p4[:], w4_sb[:], h3[:], start=True, stop=True)
                    nc.scalar.activation(zT[:, s:s + CH], p4[:], AF.Identity, bias=b4_sb[:])
                    nc.scalar.activation(zhi[:, s:s + CH], p4[:], AF.Identity, bias=b4_sb[:])
                    zt2 = hpool.tile([16, CH], f32, name="zt2")
                    nc.scalar.activation(zt2[:], p4[:], AF.Square, bias=b4_sb[:])

                    psq = psqp.tile([1, CH], f32, name="psq")
                    nc.tensor.matmul(psq[:], ones_sq[:], zt2[:], start=True, stop=True)

                    # hi/lo split of z and sq (bf16)
                    nc.vector.scalar_tensor_tensor(
                        zlo[:, s:s + CH], zT[:, s:s + CH], 0.0,
                        zhi[:, s:s + CH], Alu.add, Alu.subtract,
                    )
                    nc.scalar.activation(sqh[0:1, s:s + CH], psq[:], AF.Copy, bias=0.0)
                    nc.vector.scalar_tensor_tensor(
                        sql[0:1, s:s + CH], psq[:], 0.0,
                        sqh[0:1, s:s + CH], Alu.add, Alu.subtract,
                    )
                    nc.scalar.activation(sp1[0:1, s:s + CH], psq[:], AF.Copy, bias=1.0)

                    # bounce this chunk into the collective input buffer
                    nc.sync.dma_start(inb[0:EMB, s:s + CH], zhi[:, s:s + CH])
                    nc.sync.dma_start(inb[EMB:2 * EMB, s:s + CH], zlo[:, s:s + CH])
                    nc.sync.dma_start(inb[32:33, s:s + CH], sqh[0:1, s:s + CH])
                    nc.sync.dma_start(inb[33:34, s:s + CH], sql[0:1, s:s + CH])

            # ---- gather hi/lo splits of all points into R ----
            nc.gpsimd.collective_compute(
                "AllGather",
                mybir.AluOpType.bypass,
                replica_groups=[list(range(NCORES))],
                ins=[inb.opt()],
                outs=[outb.opt()],
            )
            for c in range(NCORES):
                nc.gpsimd.dma_start(
                    R[0:GROWS, c * ROWS:(c + 1) * ROWS],
                    outb[c * GROWS:(c + 1) * GROWS, :],
                )
                # zhi dup block, re-scattered locally
                nc.scalar.dma_start(
                    R[34:50, c * ROWS:(c + 1) * ROWS],
                    outb[c * GROWS:c * GROWS + EMB, :],
                )

            # ---- build L from own-col splits ----
            with tc.tile_pool(name="fin", bufs=1) as fin:
                m2zhi = fin.tile([EMB, ROWS], bf16)
                m2zlo = fin.tile([EMB, ROWS], bf16)
                sph = fin.tile([1, ROWS], bf16)
                spl = fin.tile([1, ROWS], bf16)

                nc.scalar.activation(m2zhi[:], zhi[:, :], AF.Copy, bias=0.0, scale=-2.0)
                nc.scalar.activation(m2zlo[:], zlo[:, :], AF.Copy, bias=0.0, scale=-2.0)
                nc.scalar.activation(sph[:], sp1[:], AF.Copy, bias=0.0)
                nc.vector.scalar_tensor_tensor(
                    spl[:], sp1[:], 0.0, sph[:], Alu.add, Alu.subtract
                )
                nc.sync.dma_start(L[0:EMB, :], m2zhi[:])
                nc.sync.dma_start(L[EMB:2 * EMB, :], m2zhi[:])
                nc.sync.dma_start(L[34:50, :], m2zlo[:])
                nc.sync.dma_start(L[50:51, :], sph[:])
                nc.sync.dma_start(L[51:52, :], spl[:])

        # ------- Phase 2: 1+dist -> recip -> rowsum -> normalize -> out -------
        W = 2048
        NW = N // W  # 4
        with (
            tc.tile_pool(name="acc", bufs=3) as apool,
            tc.tile_pool(name="acc16", bufs=2) as a16pool,
            tc.tile_pool(name="rs", bufs=2) as rspool,
            tc.tile_pool(name="psA", bufs=2, space="PSUM") as psap,
        ):
            for m in range(NCORES):
                A = apool.tile([128, N], f32, name="A")
                A16 = a16pool.tile([128, N], f16, name="A16")
                rs4 = rspool.tile([128, NW], f32, name="rs4")
                junk4 = rspool.tile([128, NW], f32, name="junk4")
                rsum = rspool.tile([128, 1], f32, name="rsum")
                rc = [
                    rspool.tile([128, 1], f32, name=f"rc{w}")
                    for w in range(1, NW)
                ]
                lm = L[:, m * 128:(m + 1) * 128]
                for w in range(NW):
                    ps = psap.tile([128, W], f32, name="ps")
                    for h in range(W // 512):
                        col = w * W + h * 512
                        nc.tensor.matmul(
                            ps[:, h * 512:(h + 1) * 512], lm,
                            R[:, col:col + 512], start=True, stop=True,
                        )
                    # num = 1/(1+dist)
                    nc.vector.reciprocal_approx_fast(
                        A[:, w * W:(w + 1) * W], ps[:]
                    )
                    # partial rowsum of this chunk (in-place copy + accum)
                    nc.scalar.activation(
                        A[:, w * W:(w + 1) * W], A[:, w * W:(w + 1) * W],
                        AF.Copy, bias=0.0, accum_out=rs4[:, w:w + 1],
                    )
                # total rowsum, plus copies (normalize_recip clobbers denom)
                nc.scalar.activation(
                    junk4[:], rs4[:], AF.Copy, bias=0.0, accum_out=rsum[:]
                )
                for w in range(1, NW):
                    nc.scalar.activation(rc[w - 1][:], rsum[:], AF.Copy, bias=0.0)
                for w in range(NW):
                    den = rsum if w == 0 else rc[w - 1]
                    nc.gpsimd.normalize_recip(
                        A16[:, w * W:(w + 1) * W], A[:, w * W:(w + 1) * W],
                        den[:],
                    )
                    nc.sync.dma_start(
                        out[m * 128:(m + 1) * 128, w * W:(w + 1) * W],
                        A16[:, w * W:(w + 1) * W],
                    )

    nc.compile()
    return nc


def _get_nc():
    if "nc" not in _CACHE:
        _CACHE["nc"] = _build_program()
    return _CACHE["nc"]


def run(inputs, trace=False):
    from concourse.bass_utils import run_bass_kernel_spmd

    nc = _get_nc()
    x = np.asarray(inputs["x"], dtype=np.float32)
    com = {
        "W1": np.ascontiguousarray(np.asarray(inputs["W1"], dtype=np.float32)),
        "W2": np.ascontiguousarray(np.asarray(inputs["W2"], dtype=np.float32)),
        "W3": np.ascontiguousarray(np.asarray(inputs["W3"], dtype=np.float32)),
        "W4": np.ascontiguousarray(np.asarray(inputs["W4"], dtype=np.float32)),
        "b1": np.ascontiguousarray(np.asarray(inputs["b1"], dtype=np.float32).reshape(-1, 1)),
        "b2": np.ascontiguousarray(np.asarray(inputs["b2"], dtype=np.float32).reshape(-1, 1)),
        "b3": np.ascontiguousarray(np.asarray(inputs["b3"], dtype=np.float32).reshape(-1, 1)),
        "b4": np.ascontiguousarray(np.asarray(inputs["b4"], dtype=np.float32).reshape(-1, 1)),
    }
    in_maps = []
    for c in range(NCORES):
        xT_c = np.ascontiguousarray(x[c * ROWS:(c + 1) * ROWS].T)
        in_maps.append({"xT": xT_c, **com})

    res = run_bass_kernel_spmd(nc, in_maps, core_ids=list(range(NCORES)), trace=trace)
    full = np.concatenate(
        [res.results[c]["out"] for c in range(NCORES)], axis=0
    ).astype(np.float32)
    return full, res


def kernel(**inputs):
    full, _ = run(inputs, trace=False)
    return full
